# revision 39
# baseline (speedup 1.0000x reference)
# Correlation2D (RAFT-style correlation pyramid lookup) on 8 TRN2 NeuronCores.
#
# Sharding: data-parallel over the bs*h*w query axis. Each core owns 1024
# queries (= 8 image rows): it computes its slice of the cost volume via
# bf16 GEMM (fmap2 replicated, pooling folded into fmap2), writes the
# 4-level pyramid per-query-contiguous to a per-block DRAM buffer, gathers
# a 10-row contiguous strip per (query, level) with one indirect DMA per
# (block, level), and does the separable bilinear combine on-chip reading
# the 10x10 patch as a strided view of the strip. Output per core is
# [1024, 324] (query-major, bf16); the host transposes to channel-major.
import numpy as np

# ---- problem constants (hardcoded per contest contract) ----
H, W = 64, 128
D = 256
NUM_LEVELS = 4
RADIUS = 4
KK = 2 * RADIUS + 1        # 9
PS = KK + 1                # 10x10 patch per (query, level)
NCORES = 8
QPC = (H * W) // NCORES    # 1024 queries per core
NBLK = QPC // 128          # 8 blocks of 128 queries
LVL_W = [W >> l for l in range(NUM_LEVELS)]            # 128 64 32 16
LVL_H = [H >> l for l in range(NUM_LEVELS)]            # 64 32 16 8
LVL_N = [LVL_W[l] * LVL_H[l] for l in range(NUM_LEVELS)]   # 8192 2048 512 128
LVL_OFF = [sum(LVL_N[:l]) for l in range(NUM_LEVELS)]  # 0 8192 10240 10752
LVLSUM = sum(LVL_N)        # 10880
PAD = 1024                 # zeroed head/tail pad (elements) of each cv buffer
BQS = 128 * LVLSUM         # cv elements per block
NCH = NUM_LEVELS * KK * KK  # 324 output channels
MM_N = 512                 # matmul N-chunk (one PSUM bank of f32)
GROUP = 2                  # N-chunks per PSUM group (2 banks)

_CACHE = {}


def _emit(ctx, tc, out_ext, f1c, f2, crd):
    import concourse.bass as bass
    import concourse.mybir as mybir

    nc = tc.nc
    f32 = mybir.dt.float32
    bf16 = mybir.dt.bfloat16
    i32 = mybir.dt.int32
    Alu = mybir.AluOpType

    const_pool = ctx.enter_context(tc.tile_pool(name="constp", bufs=1))
    f2_pool = ctx.enter_context(tc.tile_pool(name="f2p", bufs=1))
    coordp = ctx.enter_context(tc.tile_pool(name="coordp", bufs=1))
    small = ctx.enter_context(tc.tile_pool(name="small", bufs=2))
    lhsp = ctx.enter_context(tc.tile_pool(name="lhsp", bufs=2))
    cvp = ctx.enter_context(tc.tile_pool(name="cvp", bufs=2))
    stripp = ctx.enter_context(tc.tile_pool(name="stripp", bufs=2))
    txp = ctx.enter_context(tc.tile_pool(name="txp", bufs=2))
    outqp = ctx.enter_context(tc.tile_pool(name="outqp", bufs=2))
    psum = ctx.enter_context(tc.tile_pool(name="psum", bufs=1, space="PSUM"))
    dramp = ctx.enter_context(tc.tile_pool(name="dramp", bufs=1, space="DRAM"))

    # ---------------- per-block DRAM cv buffers ----------------------------
    # separate tensors give the tile scheduler precise store->gather deps
    NTOTB = PAD + BQS + PAD
    cv_dram = [
        dramp.tile([NTOTB], bf16, name=f"cv_dram{b}") for b in range(NBLK)
    ]

    # ---------------- fmap1 whole-core load --------------------------------
    # all 8 blocks' stationary tiles in one contiguous DMA (0.5 MB); loading
    # per-block strided slices costs ~14us each against the f2 stream
    f1_sb = f2_pool.tile([128, 2, NBLK * 128], bf16, name="f1_sb")
    nc.sync.dma_start(
        out=f1_sb[:],
        in_=f1c[:].rearrange("(k p) q -> p k q", k=2),
    )

    # ---------------- fmap2 load + pyramid pooling -------------------------
    # f2 as two K-halves [128 chan, 8192 pix], each split in two DMAs so the
    # first matmul group unblocks after ~3us; pooled levels keep raw SUMS,
    # the 1/16 * 0.25^l scale is folded into the PSUM drain.
    f2_lv = []
    halves = [
        f2_pool.tile([128, LVL_N[0]], bf16, name=f"f2h{k}") for k in range(2)
    ]
    # first pieces small so the first matmul group unblocks ASAP behind f1
    PIECES = [(0, 1024), (1024, 2048), (2048, 4096), (4096, 8192)]
    for (p0, p1) in PIECES:
        for k in range(2):
            nc.scalar.dma_start(
                out=halves[k][:, p0:p1],
                in_=f2[k * 128 : (k + 1) * 128, p0:p1],
            )
    f2_lv.append(halves)
    for l in range(1, NUM_LEVELS):
        Wl, Hl = LVL_W[l], LVL_H[l]
        pw, ph = LVL_W[l - 1], LVL_H[l - 1]
        halves = []
        for k in range(2):
            prev = f2_lv[l - 1][k][:].rearrange(
                "p (h w two) -> p h w two", h=ph, w=pw // 2, two=2
            )
            s1 = small.tile(
                [128, ph, pw // 2], bf16, name=f"s1_{l}_{k}", tag="poolt", bufs=1
            )
            nc.vector.tensor_tensor(
                s1[:], prev[:, :, :, 0], prev[:, :, :, 1], op=Alu.add
            )
            s1v = s1[:].rearrange("p (h2 two) w -> p h2 two w", h2=Hl, two=2)
            cur = f2_pool.tile([128, Hl * Wl], bf16, name=f"f2l{l}_{k}")
            curv = cur[:].rearrange("p (h w) -> p h w", h=Hl, w=Wl)
            nc.vector.tensor_tensor(
                curv[:], s1v[:, :, 0, :], s1v[:, :, 1, :], op=Alu.add
            )
            halves.append(cur)
        f2_lv.append(halves)

    # ---------------- coords -> indices / weights --------------------------
    # level-vectorized: tiles carry a trailing NUM_LEVELS axis
    cxs = coordp.tile([128, NBLK], f32, name="cxs")
    cys = coordp.tile([128, NBLK], f32, name="cys")
    nc.scalar.dma_start(out=cxs[:], in_=crd[0, :].rearrange("(b p) -> p b", p=128))
    nc.scalar.dma_start(out=cys[:], in_=crd[1, :].rearrange("(b p) -> p b", p=128))

    # zeros for cv pad regions (garbage there is gathered but must stay
    # finite; it is multiplied by a zero weight)
    ztile = const_pool.tile([128, 8], bf16, name="ztile")
    nc.vector.memset(ztile[:], 0.0)

    def emit_pads(b):
        # zero this block's cv head/tail pads on the SWDGE queue just ahead
        # of its gathers; gpsimd is otherwise idle
        nc.gpsimd.dma_start(
            out=cv_dram[b][0:PAD].rearrange("(p x) -> p x", p=128),
            in_=ztile[:],
        )
        nc.gpsimd.dma_start(
            out=cv_dram[b][PAD + BQS : PAD + BQS + PAD].rearrange(
                "(p x) -> p x", p=128
            ),
            in_=ztile[:],
        )

    # per-level constants [128, 1, NUM_LEVELS] (broadcast over b)
    invsc = const_pool.tile([128, 1, NUM_LEVELS], f32, name="invsc")
    wlv = const_pool.tile([128, 1, NUM_LEVELS], f32, name="wlv")
    offv = const_pool.tile([128, 1, NUM_LEVELS], f32, name="offv")
    limx = const_pool.tile([128, 1, NUM_LEVELS], f32, name="limx")
    limy = const_pool.tile([128, 1, NUM_LEVELS], f32, name="limy")
    crampf = const_pool.tile([128, PS], f32, name="crampf")
    bqf = coordp.tile([128, 1], f32, name="bqf")

    shp = [128, NBLK, NUM_LEVELS]
    idx_i = coordp.tile(shp, i32, name="idx_i")
    wx0e = coordp.tile([128, NBLK, NUM_LEVELS, KK], bf16, name="wx0e")
    wx1e = coordp.tile([128, NBLK, NUM_LEVELS, KK], bf16, name="wx1e")
    wy0e = coordp.tile([128, NBLK, NUM_LEVELS, KK], bf16, name="wy0e")
    wy1e = coordp.tile([128, NBLK, NUM_LEVELS, KK], bf16, name="wy1e")
    ff_fr = {}

    def floor_frac(src, nm):
        """src [128,NBLK] f32 coords -> (floor f32, frac f32) per level."""
        xs = small.tile(shp, f32, name=f"xs_{nm}", tag="xs")
        nc.vector.tensor_tensor(
            xs[:],
            src[:].unsqueeze(2).to_broadcast(shp),
            invsc[:].to_broadcast(shp),
            op=Alu.mult,
        )
        ii = small.tile(shp, i32, name=f"ii_{nm}", tag="ii")
        nc.vector.tensor_copy(out=ii[:], in_=xs[:])          # f32 -> i32 cast
        ff = coordp.tile(shp, f32, name=f"ff_{nm}")
        nc.vector.tensor_copy(out=ff[:], in_=ii[:])          # back to f32
        adj = small.tile(shp, f32, name=f"adj_{nm}", tag="adj")
        nc.vector.tensor_tensor(adj[:], ff[:], xs[:], op=Alu.is_gt)
        nc.vector.tensor_tensor(ff[:], ff[:], adj[:], op=Alu.subtract)  # floor
        fr = coordp.tile(shp, f32, name=f"fr_{nm}")
        nc.vector.tensor_tensor(fr[:], xs[:], ff[:], op=Alu.subtract)   # frac
        return ff, fr

    def emit_idx_prep():
        eng_load["v"] += 3000.0
        # consts needed for the gather index math
        for l in range(NUM_LEVELS):
            nc.vector.memset(invsc[:, :, l], 1.0 / (1 << l))
            nc.vector.memset(wlv[:, :, l], float(LVL_W[l]))
            nc.vector.memset(
                offv[:, :, l], float(LVL_OFF[l] - RADIUS * LVL_W[l] - RADIUS + PAD)
            )
        # per-query element base offset within its block's cv buffer: p*LVLSUM
        bq_i = small.tile([128, 1], i32, name="bq_i", tag="bq_i")
        nc.gpsimd.iota(bq_i[:], pattern=[[1, 1]], base=0, channel_multiplier=1)
        nc.vector.tensor_copy(out=bqf[:], in_=bq_i[:])
        nc.vector.tensor_scalar_mul(bqf[:], bqf[:], float(LVLSUM))

        ff_fr["x"] = floor_frac(cxs, "x")
        ff_fr["y"] = floor_frac(cys, "y")
        ixf, iyf = ff_fr["x"][0], ff_fr["y"][0]

        # gather start index: bq + lvl_off + (iy-4)*Wl + (ix-4) + PAD
        t1 = small.tile(shp, f32, name="t1", tag="t1")
        nc.vector.tensor_tensor(
            t1[:], iyf[:], wlv[:].to_broadcast(shp), op=Alu.mult
        )
        nc.vector.tensor_tensor(t1[:], t1[:], ixf[:], op=Alu.add)
        nc.vector.tensor_tensor(
            t1[:], t1[:], bqf[:].unsqueeze(2).to_broadcast(shp), op=Alu.add
        )
        nc.vector.tensor_tensor(
            t1[:], t1[:], offv[:].to_broadcast(shp), op=Alu.add
        )
        nc.vector.tensor_copy(out=idx_i[:], in_=t1[:])  # exact ints

    def emit_weights_prep():
        eng_load["v"] += 5000.0
        # interp weights with the OOB zero-mask folded in
        for l in range(NUM_LEVELS):
            nc.vector.memset(limx[:, :, l], float(LVL_W[l] - 1))
            nc.vector.memset(limy[:, :, l], float(LVL_H[l] - 1))
        # c ramp: -4..5 (patch-col -> absolute offset from floor(coord))
        cramp_i = small.tile([128, PS], i32, name="cramp_i", tag="cramp_i")
        nc.gpsimd.iota(
            cramp_i[:], pattern=[[1, PS]], base=-RADIUS, channel_multiplier=0
        )
        nc.vector.tensor_copy(out=crampf[:], in_=cramp_i[:])

        shp4 = [128, NBLK, NUM_LEVELS, PS]
        shpk = [128, NBLK, NUM_LEVELS, KK]
        for (w0t, w1t), (posf, frac), lim in (
            ((wx0e, wx1e), ff_fr["x"], limx),
            ((wy0e, wy1e), ff_fr["y"], limy),
        ):
            # tap positions posf-4+c for c in 0..9
            pos = small.tile(shp4, f32, name="pos", tag="pos")
            nc.vector.tensor_tensor(
                pos[:],
                posf[:].unsqueeze(3).to_broadcast(shp4),
                crampf[:].unsqueeze(1).unsqueeze(1).to_broadcast(shp4),
                op=Alu.add,
            )
            # in-bounds <=> |2*pos - lim| <= lim
            nc.vector.tensor_scalar_mul(pos[:], pos[:], 2.0)
            nc.vector.tensor_tensor(
                pos[:],
                pos[:],
                lim[:].unsqueeze(3).to_broadcast(shp4),
                op=Alu.subtract,
            )
            ok = small.tile(shp4, f32, name="ok", tag="ok")
            nc.scalar.activation(ok[:], pos[:], mybir.ActivationFunctionType.Abs)
            nc.vector.tensor_tensor(
                ok[:], ok[:], lim[:].unsqueeze(3).to_broadcast(shp4), op=Alu.is_le
            )
            w0 = small.tile(shp, f32, name="w0", tag="w0")
            nc.vector.tensor_scalar(w0[:], frac[:], -1.0, 1.0,
                                    op0=Alu.mult, op1=Alu.add)  # 1 - frac
            nc.vector.tensor_tensor(
                w0t[:],
                w0[:].unsqueeze(3).to_broadcast(shpk),
                ok[:, :, :, 0:KK],
                op=Alu.mult,
            )
            nc.vector.tensor_tensor(
                w1t[:],
                frac[:].unsqueeze(3).to_broadcast(shpk),
                ok[:, :, :, 1:PS],
                op=Alu.mult,
            )

    # ---------------- per-block pipeline -----------------------------------
    # chunk schedule: groups of <=GROUP N-chunks sharing one PSUM tile
    groups = []
    for l in range(NUM_LEVELS):
        for n0 in range(0, LVL_N[l], MM_N * GROUP):
            g = []
            for c in range(GROUP):
                a = n0 + c * MM_N
                if a >= LVL_N[l]:
                    break
                g.append((l, a, min(LVL_N[l], a + MM_N)))
            groups.append(g)
    # merge trailing small groups (l2 + l3 fit in one PSUM tile)
    merged = []
    for g in groups:
        if merged and sum(b - a for _, a, b in merged[-1]) + sum(
            b - a for _, a, b in g
        ) <= MM_N * GROUP and len(merged[-1]) + len(g) <= GROUP:
            merged[-1] = merged[-1] + g
        else:
            merged.append(g)
    groups = merged

    # drain-engine balancing: vector also runs the combine and coord prep,
    # charged to eng_load as those are emitted; one 8-bank PSUM tile, groups
    # rotate through quarter slices
    eng_load = {"v": 0.0, "s": 0.0}
    PSUM_W = 4096
    pt_all = psum.tile([128, PSUM_W], f32, name="pt_all")
    gi_state = {"gi": 0}

    def drain_cost(n):
        return 318.0 + 0.714 * n

    def emit_gemm_block(b):
        cv_sb = [
            cvp.tile(
                [128, LVL_N[l]], bf16, name=f"cv{l}", tag=f"cv{l}", bufs=3
            )
            for l in range(NUM_LEVELS)
        ]
        cvq = cv_dram[b][PAD : PAD + BQS].rearrange("(q s) -> q s", s=LVLSUM)
        drained = [0] * NUM_LEVELS   # contiguous drained extent per level
        stored = [0] * NUM_LEVELS    # stored extent per level

        def maybe_store(l, final=False):
            # store drained cv in >=2048-col chunks as soon as available so
            # the gathers start right after the block's last drain
            hi = drained[l]
            if final:
                hi = LVL_N[l]
            ready = hi - stored[l]
            if ready >= 2048 or (hi == LVL_N[l] and ready > 0):
                nc.sync.dma_start(
                    out=cvq[
                        :, LVL_OFF[l] + stored[l] : LVL_OFF[l] + hi
                    ],
                    in_=cv_sb[l][:, stored[l] : hi],
                )
                stored[l] = hi

        for g in groups:
            base = (gi_state["gi"] * MM_N * GROUP) % PSUM_W
            gi_state["gi"] += 1
            pt = pt_all[:, base : base + MM_N * GROUP]
            for k in range(2):
                o = 0
                for (l, a, bb) in g:
                    nc.tensor.matmul(
                        pt[:, o : o + bb - a],
                        f1_sb[:, k, b * 128 : (b + 1) * 128],
                        f2_lv[l][k][:, a:bb],
                        start=(k == 0),
                        stop=(k == 1),
                    )
                    o += bb - a
            # drain: one op per contiguous (level, range) span in the group,
            # greedily balanced between vector and scalar engines
            o = 0
            spans = []
            for (l, a, bb) in g:
                if spans and spans[-1][0] == l and spans[-1][2] == a:
                    spans[-1] = (l, spans[-1][1], bb, spans[-1][3])
                else:
                    spans.append((l, a, bb, o))
                o += bb - a
            for (l, a, bb, o) in spans:
                scale_l = (1.0 / 16.0) * (0.25 ** l)
                dst = cv_sb[l][:, a:bb]
                if eng_load["v"] <= eng_load["s"]:
                    nc.vector.tensor_scalar_mul(dst[:], pt[:, o : o + bb - a], scale_l)
                    eng_load["v"] += drain_cost(bb - a)
                else:
                    nc.scalar.mul(dst[:], pt[:, o : o + bb - a], scale_l)
                    eng_load["s"] += drain_cost(bb - a)
                drained[l] = max(drained[l], bb)
                maybe_store(l)
        for l in range(NUM_LEVELS):
            maybe_store(l, final=True)

    strip_store = {}

    def emit_gather_block(b):
        # gather strips: one indirect DMA per level, one offset per query
        # (HW contract: one offset per dest partition row, contiguous run)
        strips = [
            stripp.tile(
                [128, PS * LVL_W[l]], bf16, name=f"strip{l}", tag=f"strip{l}"
            )
            for l in range(NUM_LEVELS)
        ]
        strip_store[b] = strips
        cv2d = cv_dram[b][:].rearrange("(a x) -> a x", x=NTOTB // 1024)
        for l in range(NUM_LEVELS):
            nc.gpsimd.indirect_dma_start(
                out=strips[l][:],
                out_offset=None,
                in_=cv2d,
                in_offset=bass.IndirectOffsetOnAxis(
                    ap=idx_i[:, b, l].unsqueeze(1), axis=1
                ),
            )

    def emit_combine_block(b):
        eng_load["v"] += 6000.0
        # separable bilinear: x-combine per level (strided strip views),
        # then y-combine in f32 for output precision
        strips = strip_store.pop(b)
        tx = txp.tile([128, NUM_LEVELS, PS, KK], bf16, name="tx", tag="tx")
        tx2 = txp.tile([128, NUM_LEVELS, PS, KK], bf16, name="tx2", tag="tx2")
        for l in range(NUM_LEVELS):
            pv = strips[l][:].rearrange("p (r w) -> p r w", r=PS, w=LVL_W[l])
            bshape_l = [128, PS, KK]
            nc.vector.tensor_tensor(
                tx[:, l, :, :], pv[:, :, 0:KK],
                wx0e[:, b, l, :].unsqueeze(1).to_broadcast(bshape_l),
                op=Alu.mult,
            )
            nc.vector.tensor_tensor(
                tx2[:, l, :, :], pv[:, :, 1:PS],
                wx1e[:, b, l, :].unsqueeze(1).to_broadcast(bshape_l),
                op=Alu.mult,
            )
        nc.vector.tensor_tensor(tx[:], tx[:], tx2[:], op=Alu.add)

        outq = outqp.tile([128, NUM_LEVELS, KK, KK], f32, name="outq", tag="outq")
        outq2 = outqp.tile(
            [128, NUM_LEVELS, KK, KK], f32, name="outq2", tag="outq2"
        )
        bshape_y = [128, NUM_LEVELS, KK, KK]
        nc.vector.tensor_tensor(
            outq[:], tx[:, :, 0:KK, :],
            wy0e[:, b, :, :].unsqueeze(3).to_broadcast(bshape_y), op=Alu.mult,
        )
        nc.vector.tensor_tensor(
            outq2[:], tx[:, :, 1:PS, :],
            wy1e[:, b, :, :].unsqueeze(3).to_broadcast(bshape_y), op=Alu.mult,
        )
        nc.vector.tensor_tensor(outq[:], outq[:], outq2[:], op=Alu.add)

        # store query-major output; host transposes to channel-major
        nc.sync.dma_start(
            out=out_ext[b * 128 : (b + 1) * 128, :],
            in_=outq[:].rearrange("p l dy dx -> p (l dy dx)"),
        )

    # software-pipelined emission: coord prep interleaves with the first two
    # blocks' GEMMs so the first PSUM drains aren't queued behind it; the
    # combine for block b is emitted one block late so its strips are ready.
    for b in range(NBLK):
        emit_pads(b)
    for b in range(NBLK):
        emit_gemm_block(b)
        if b == 0:
            emit_idx_prep()
        if b == 1:
            emit_weights_prep()
        emit_gather_block(b)
        if b >= 1:
            emit_combine_block(b - 1)
    emit_combine_block(NBLK - 1)


def build_program():
    """Build (once) the single-core SPMD bass program."""
    key = "nc"
    if key in _CACHE:
        return _CACHE[key]
    import concourse.tile as tile
    import concourse.mybir as mybir
    from concourse import bacc

    f32 = mybir.dt.float32
    bf16 = mybir.dt.bfloat16
    nc = bacc.Bacc(
        "TRN2",
        target_bir_lowering=False,
        debug=False,
        enable_asserts=True,
        num_devices=NCORES,
    )
    f1c = nc.dram_tensor("f1c", [D, QPC], bf16, kind="ExternalInput").ap()
    f2 = nc.dram_tensor("f2", [D, H * W], bf16, kind="ExternalInput").ap()
    crd = nc.dram_tensor("crd", [2, QPC], f32, kind="ExternalInput").ap()
    out = nc.dram_tensor("out", [QPC, NCH], f32, kind="ExternalOutput").ap()

    from contextlib import ExitStack

    with tile.TileContext(nc) as tc, ExitStack() as ctx:
        _emit(ctx, tc, out, f1c, f2, crd)
    nc.compile()
    _CACHE[key] = nc
    return nc


def make_in_maps(fmap1, fmap2, coords):
    import ml_dtypes

    bf16 = ml_dtypes.bfloat16
    f1 = np.asarray(fmap1, dtype=np.float32).reshape(D, H * W).astype(bf16)
    f2 = np.ascontiguousarray(
        np.asarray(fmap2, dtype=np.float32).reshape(D, H * W).astype(bf16)
    )
    crd = np.asarray(coords, dtype=np.float32).reshape(2, H * W)
    in_maps = []
    for c in range(NCORES):
        sl = slice(c * QPC, (c + 1) * QPC)
        in_maps.append(
            {
                "f1c": np.ascontiguousarray(f1[:, sl]),
                "f2": f2,
                "crd": np.ascontiguousarray(crd[:, sl]),
            }
        )
    return in_maps


def kernel(fmap1, fmap2, coords):
    from concourse.bass_utils import run_bass_kernel_spmd

    nc = build_program()
    in_maps = make_in_maps(fmap1, fmap2, coords)
    res = run_bass_kernel_spmd(nc, in_maps, list(range(NCORES)))
    parts = [res.results[c]["out"] for c in range(NCORES)]  # [1024, 324] each
    full = np.concatenate(parts, axis=0).astype(np.float32)  # [4096, 324]
    full = full.reshape(1, H, W, NCH).transpose(0, 3, 1, 2)  # [1, 324, 64, 128]
    return np.ascontiguousarray(full)
